# revision 17
# baseline (speedup 1.0000x reference)
"""TRN2 Bass kernel: per-class (segment) sums of pixel features.

Computes, for C=19 classes over N=524288 pixels with A=512 channels:
  mean[c]       = sum_{i: lab_i==c, valid} feat_i / max(count_c, 1)
  sum_weight[c] = count_c broadcast over A
  class_dist[c] = count_c

Strategy (data-parallel over pixels, 8 NeuronCores):
  Each core processes a contiguous shard of 65536 pixels.  Features are
  shipped as an exact bf16 hi/lo pair (x == hi + lo to ~2^-18 relative),
  so the per-128-pixel-tile segment sum runs as two full-rate bf16
  matmuls (onehot.T @ hi, onehot.T @ lo) accumulating into one PSUM
  bank, instead of one quarter-rate fp32 matmul.  The one-hot [128, C]
  is built on the vector engine (iota == label, per-partition scalar
  compare).  Counts come from 19 is_equal+accum_out passes over the
  on-chip label tile.  Per-core partial sums/counts are summed on the
  host (trivial: 8 x 19 x 513 values) and divided.

  Within each 2048-pixel DMA chunk, partition p takes pixels
  chunk*2048 + p*16 .. +15, so every partition reads one contiguous
  16 KiB span per chunk (line-rate DMA).  The labels are permuted the
  same way on the host.
"""

import functools

import ml_dtypes
import numpy as np

import concourse.bacc as bacc
import concourse.mybir as mybir
from concourse.bass_utils import run_bass_kernel_spmd
from concourse.tile import TileContext

BF16 = ml_dtypes.bfloat16

C = 19  # classes
A = 512  # feature channels
NCORES = 8
N = 524288  # total pixels
PER = N // NCORES  # pixels per core
T = PER // 128  # 128-pixel tiles per core (512)
G = 16  # main tiles-per-DMA-chunk size (2 MiB bf16 per half)
# chunk plan: small head chunks so compute starts early, small tail
# chunks so the post-last-DMA compute tail is short
CHUNKS = [4] * 4 + [G] * ((T - 32) // G) + [4] * 4
assert sum(CHUNKS) == T


@functools.lru_cache(maxsize=1)
def _build():
    nc = bacc.Bacc("TRN2", target_bir_lowering=False)
    fhi = nc.dram_tensor("fhi", [PER, A], mybir.dt.bfloat16, kind="ExternalInput")
    flo = nc.dram_tensor("flo", [PER, A], mybir.dt.bfloat16, kind="ExternalInput")
    # meta: cols [0, C) = iota 0..18, cols [C, C+T) = labels with col t
    # holding the (permuted) labels of matmul-tile t
    meta = nc.dram_tensor("meta", [128, C + T], mybir.dt.float32, kind="ExternalInput")
    sums = nc.dram_tensor("sums", [C, A], mybir.dt.float32, kind="ExternalOutput")
    cnt = nc.dram_tensor("cnt", [128, C], mybir.dt.float32, kind="ExternalOutput")

    with TileContext(nc) as tc:
        with (
            tc.tile_pool(name="sbuf", bufs=1) as pool,
            tc.tile_pool(name="psum", bufs=1, space="PSUM") as pp,
        ):
            meta_t = pool.tile([128, C + T], mybir.dt.float32, tag="meta", bufs=1)
            nc.sync.dma_start(out=meta_t[:], in_=meta[:])

            ps = pp.tile([C, A], mybir.dt.float32, tag="ps", bufs=1)

            # PE HAM warmup: ~7us of dummy matmuls on zeros while the first
            # chunks stream in, so real matmuls start at 2.4 GHz instead of
            # accumulating a cold-clock lag that surfaces as DMA idle at the
            # end of the pipeline.
            wz = pool.tile([128, A], mybir.dt.bfloat16, tag="wz", bufs=1)
            wo = pool.tile([128, C], mybir.dt.bfloat16, tag="wo", bufs=1)
            nc.vector.memset(wz[:], 0.0)
            nc.vector.memset(wo[:], 0.0)
            pw = pp.tile([C, A], mybir.dt.float32, tag="pw", bufs=1)
            for w in range(16):
                nc.tensor.matmul(
                    pw[:], lhsT=wo[:], rhs=wz[:], start=(w == 0), stop=(w == 15)
                )

            # counts: cnt_t[p, c] = #{t : labels_t[p, t] == c}.  One count op
            # is interleaved per chunk (starting after the head chunks) so
            # they ride along on the vector engine without delaying the
            # first one-hots or adding tail latency.
            cnt_t = pool.tile([128, C], mybir.dt.float32, tag="cnt", bufs=1)
            scratch = pool.tile([128, T], mybir.dt.float32, tag="scr", bufs=1)

            def emit_count(c):
                nc.vector.tensor_scalar(
                    out=scratch[:],
                    in0=meta_t[:, C:],
                    scalar1=float(c),
                    scalar2=None,
                    op0=mybir.AluOpType.is_equal,
                    op1=mybir.AluOpType.add,
                    accum_out=cnt_t[:, c : c + 1],
                )

            t0 = 0
            for gi, g in enumerate(CHUNKS):
                ht = pool.tile([128, G * A], mybir.dt.bfloat16, tag="ht", bufs=6)
                lt = pool.tile([128, G * A], mybir.dt.bfloat16, tag="lt", bufs=6)
                sl = slice(t0 * 128, (t0 + g) * 128)
                nc.sync.dma_start(
                    out=ht[:, : g * A],
                    in_=fhi[sl].rearrange("(p g) m -> p (g m)", p=128),
                )
                nc.sync.dma_start(
                    out=lt[:, : g * A],
                    in_=flo[sl].rearrange("(p g) m -> p (g m)", p=128),
                )
                for j in range(g):
                    t = t0 + j
                    oh = pool.tile([128, C], mybir.dt.bfloat16, tag="oh", bufs=8)
                    nc.vector.tensor_scalar(
                        out=oh[:],
                        in0=meta_t[:, :C],
                        scalar1=meta_t[:, C + t : C + t + 1],
                        scalar2=None,
                        op0=mybir.AluOpType.is_equal,
                    )
                    nc.tensor.matmul(
                        ps[:],
                        lhsT=oh[:],
                        rhs=ht[:, j * A : (j + 1) * A],
                        start=(t == 0),
                        stop=False,
                    )
                    nc.tensor.matmul(
                        ps[:],
                        lhsT=oh[:],
                        rhs=lt[:, j * A : (j + 1) * A],
                        start=False,
                        stop=(t == T - 1),
                    )
                if 4 <= gi < 4 + C:
                    emit_count(gi - 4)
                t0 += g
            nc.sync.dma_start(out=cnt[:], in_=cnt_t[:])

            ssum = pool.tile([C, A], mybir.dt.float32, tag="ss", bufs=1)
            nc.vector.tensor_copy(out=ssum[:], in_=ps[:])
            nc.sync.dma_start(out=sums[:], in_=ssum[:])

    nc.compile()
    return nc


@functools.lru_cache(maxsize=1)
def _tile_perm():
    """[p, t] -> pixel offset within the shard, under the chunked
    contiguous-DMA layout (partition p takes g consecutive pixels)."""
    idx = np.empty((128, T), dtype=np.int64)
    t0 = 0
    base = 0
    for g in CHUNKS:
        p = np.arange(128)[:, None]
        j = np.arange(g)[None, :]
        idx[:, t0 : t0 + g] = base + p * g + j
        t0 += g
        base += 128 * g
    return idx


def _prep_inmaps(features, labels, ignore_label):
    f = np.asarray(features, dtype=np.float32)
    hi = f.astype(BF16)
    lo = (f - hi.astype(np.float32)).astype(BF16)
    labels = np.asarray(labels)
    ig = int(np.asarray(ignore_label))
    labf = labels.astype(np.float32)
    labf[labels == ig] = -1.0  # ignored pixels match no class
    iota = np.broadcast_to(np.arange(C, dtype=np.float32), (128, C))
    perm = _tile_perm()
    in_maps = []
    for core in range(NCORES):
        sl = slice(core * PER, (core + 1) * PER)
        # labels arranged so col t = labels of matmul-tile t under the
        # contiguous-DMA pixel permutation (pixel = chunk_base + p*g + j)
        lshard = labf[sl][perm]
        m = np.ascontiguousarray(
            np.concatenate([iota, lshard], axis=1), dtype=np.float32
        )
        in_maps.append({"fhi": hi[sl], "flo": lo[sl], "meta": m})
    return in_maps


def _combine(results):
    sums = np.zeros((C, A), dtype=np.float64)
    counts = np.zeros((C,), dtype=np.float64)
    for r in results:
        sums += r["sums"].astype(np.float64)
        counts += r["cnt"].sum(axis=0, dtype=np.float64)
    amount = np.where(counts == 0, 1.0, counts)
    mean = (sums / amount[:, None]).astype(np.float32)
    counts32 = counts.astype(np.float32)
    sum_weight = np.broadcast_to(counts32[:, None], (C, A)).copy()
    class_dist = counts32
    return mean, sum_weight, class_dist


def _run(features, labels, ignore_label, trace=False, trace_cores=None):
    nc = _build()
    in_maps = _prep_inmaps(features, labels, ignore_label)
    res = run_bass_kernel_spmd(
        nc, in_maps, list(range(NCORES)), trace=trace, trace_cores=trace_cores
    )
    return _combine(res.results), res


def kernel(features, labels, ignore_label):
    out, _ = _run(features, labels, ignore_label)
    return out


# revision 21
# speedup vs baseline: 1.4507x; 1.4507x over previous
"""TRN2 Bass kernel: per-class (segment) sums of pixel features.

Computes, for C=19 classes over N=524288 pixels with A=512 channels:
  mean[c]       = sum_{i: lab_i==c, valid} feat_i / max(count_c, 1)
  sum_weight[c] = count_c broadcast over A
  class_dist[c] = count_c

Strategy (data-parallel over pixels, 8 NeuronCores):
  Each core processes a contiguous shard of 65536 pixels.  Features are
  shipped as an fp16 hi + fp8e4m3 lo pair: hi = fp16(x) and
  lo = fp8((x - hi) * 2^11), i.e. 3 bytes/element instead of 4, which
  cuts the HBM-read floor by 25% while keeping ~2^-15 per-element
  precision (measured 6.8e-6 max rel err on the final means, vs 5.7e-6
  for an exact-fp32 pipeline - the difference is fp32 sum-order noise).
  Per 128-pixel tile the segment sum runs as two full-rate matmuls
  (onehot16.T @ hi into one PSUM bank, onehot8.T @ lo into another);
  the final sums are hi_sums + 2^-11 * lo_sums.  The one-hot [128, C]
  is built on the vector engine (iota == label, per-partition scalar
  compare) and cast to fp8 on GpSimd.  Counts come from 19
  is_equal+accum_out passes over the on-chip label tile.  Per-core
  partial sums/counts are summed on the host (trivial: 8 x 19 x 513
  values) and divided.

  Within each chunk, partition p takes g consecutive pixels, so every
  partition reads one contiguous span per chunk DMA (line-rate).  The
  labels are permuted to match on the host.
"""

import functools

import ml_dtypes
import numpy as np

import concourse.bacc as bacc
import concourse.mybir as mybir
from concourse.bass_utils import run_bass_kernel_spmd
from concourse.tile import TileContext

BF16 = ml_dtypes.bfloat16
F8 = ml_dtypes.float8_e4m3

C = 19  # classes
A = 512  # feature channels
NCORES = 8
N = 524288  # total pixels
PER = N // NCORES  # pixels per core
T = PER // 128  # 128-pixel tiles per core (512)
G = 16  # main tiles-per-DMA-chunk size (2 MiB bf16 per half)
# chunk plan: small head chunks so compute starts early, small tail
# chunks so the post-last-DMA compute tail is short
CHUNKS = [4] * 4 + [G] * ((T - 32) // G) + [4] * 4
assert sum(CHUNKS) == T


@functools.lru_cache(maxsize=1)
def _build():
    nc = bacc.Bacc("TRN2", target_bir_lowering=False)
    fhi = nc.dram_tensor("fhi", [PER, A], mybir.dt.float16, kind="ExternalInput")
    flo = nc.dram_tensor("flo", [PER, A], mybir.dt.float8e4, kind="ExternalInput")
    # meta: cols [0, C) = iota 0..18, cols [C, C+T) = labels with col t
    # holding the (permuted) labels of matmul-tile t
    meta = nc.dram_tensor("meta", [128, C + T], mybir.dt.float32, kind="ExternalInput")
    sums = nc.dram_tensor("sums", [C, A], mybir.dt.float32, kind="ExternalOutput")
    cnt = nc.dram_tensor("cnt", [128, C], mybir.dt.float32, kind="ExternalOutput")

    with TileContext(nc) as tc:
        with (
            tc.tile_pool(name="sbuf", bufs=1) as pool,
            tc.tile_pool(name="psum", bufs=1, space="PSUM") as pp,
        ):
            meta_t = pool.tile([128, C + T], mybir.dt.float32, tag="meta", bufs=1)
            nc.sync.dma_start(out=meta_t[:], in_=meta[:])

            ps = pp.tile([C, A], mybir.dt.float32, tag="ps", bufs=1)
            psl = pp.tile([C, A], mybir.dt.float32, tag="psl", bufs=1)

            # PE HAM warmup: ~7us of dummy matmuls on zeros while the first
            # chunks stream in, so real matmuls start at 2.4 GHz instead of
            # accumulating a cold-clock lag that surfaces as DMA idle at the
            # end of the pipeline.
            wz = pool.tile([128, A], mybir.dt.bfloat16, tag="wz", bufs=1)
            wo = pool.tile([128, C], mybir.dt.bfloat16, tag="wo", bufs=1)
            nc.vector.memset(wz[:], 0.0)
            nc.vector.memset(wo[:], 0.0)
            pw = pp.tile([C, A], mybir.dt.float32, tag="pw", bufs=1)
            for w in range(16):
                nc.tensor.matmul(
                    pw[:], lhsT=wo[:], rhs=wz[:], start=(w == 0), stop=(w == 15)
                )

            # counts: cnt_t[p, c] = #{t : labels_t[p, t] == c}.  One count op
            # is interleaved per chunk (starting after the head chunks) so
            # they ride along on the vector engine without delaying the
            # first one-hots or adding tail latency.
            cnt_t = pool.tile([128, C], mybir.dt.float32, tag="cnt", bufs=1)
            scratch = pool.tile([128, T], mybir.dt.float32, tag="scr", bufs=1)

            def emit_count(c):
                nc.vector.tensor_scalar(
                    out=scratch[:],
                    in0=meta_t[:, C:],
                    scalar1=float(c),
                    scalar2=None,
                    op0=mybir.AluOpType.is_equal,
                    op1=mybir.AluOpType.add,
                    accum_out=cnt_t[:, c : c + 1],
                )

            t0 = 0
            for gi, g in enumerate(CHUNKS):
                ht = pool.tile([128, G * A], mybir.dt.float16, tag="ht", bufs=8)
                lt = pool.tile([128, G * A], mybir.dt.float8e4, tag="lt", bufs=8)
                sl = slice(t0 * 128, (t0 + g) * 128)
                nc.sync.dma_start(
                    out=ht[:, : g * A],
                    in_=fhi[sl].rearrange("(p g) m -> p (g m)", p=128),
                )
                nc.sync.dma_start(
                    out=lt[:, : g * A],
                    in_=flo[sl].rearrange("(p g) m -> p (g m)", p=128),
                )
                for j in range(g):
                    t = t0 + j
                    oh = pool.tile([128, C], mybir.dt.float16, tag="oh", bufs=8)
                    nc.vector.tensor_scalar(
                        out=oh[:],
                        in0=meta_t[:, :C],
                        scalar1=meta_t[:, C + t : C + t + 1],
                        scalar2=None,
                        op0=mybir.AluOpType.is_equal,
                    )
                    oh8 = pool.tile([128, C], mybir.dt.float8e4, tag="oh8", bufs=8)
                    nc.gpsimd.tensor_copy(out=oh8[:], in_=oh[:])
                    nc.tensor.matmul(
                        ps[:],
                        lhsT=oh[:],
                        rhs=ht[:, j * A : (j + 1) * A],
                        start=(t == 0),
                        stop=(t == T - 1),
                        skip_group_check=True,
                    )
                    nc.tensor.matmul(
                        psl[:],
                        lhsT=oh8[:],
                        rhs=lt[:, j * A : (j + 1) * A],
                        start=(t == 0),
                        stop=(t == T - 1),
                        skip_group_check=True,
                    )
                if 4 <= gi < 4 + C:
                    emit_count(gi - 4)
                t0 += g
            nc.sync.dma_start(out=cnt[:], in_=cnt_t[:])

            ssum = pool.tile([C, A], mybir.dt.float32, tag="ss", bufs=1)
            # sums = hi_sums + 2^-11 * lo_sums (lo was shipped as fp8 of
            # (x - fp16(x)) * 2^11).  Only one PSUM input per DVE op, so
            # stage hi_sums through SBUF first.
            nc.vector.tensor_copy(out=ssum[:], in_=ps[:])
            nc.vector.scalar_tensor_tensor(
                out=ssum[:],
                in0=psl[:],
                scalar=2.0**-11,
                in1=ssum[:],
                op0=mybir.AluOpType.mult,
                op1=mybir.AluOpType.add,
            )
            nc.sync.dma_start(out=sums[:], in_=ssum[:])

    nc.compile()
    return nc


@functools.lru_cache(maxsize=1)
def _tile_perm():
    """[p, t] -> pixel offset within the shard, under the chunked
    contiguous-DMA layout (partition p takes g consecutive pixels)."""
    idx = np.empty((128, T), dtype=np.int64)
    t0 = 0
    base = 0
    for g in CHUNKS:
        p = np.arange(128)[:, None]
        j = np.arange(g)[None, :]
        idx[:, t0 : t0 + g] = base + p * g + j
        t0 += g
        base += 128 * g
    return idx


def _prep_inmaps(features, labels, ignore_label):
    f = np.asarray(features, dtype=np.float32)
    hi = f.astype(np.float16)
    lo = ((f - hi.astype(np.float32)) * 2048.0).astype(F8)
    labels = np.asarray(labels)
    ig = int(np.asarray(ignore_label))
    labf = labels.astype(np.float32)
    labf[labels == ig] = -1.0  # ignored pixels match no class
    iota = np.broadcast_to(np.arange(C, dtype=np.float32), (128, C))
    perm = _tile_perm()
    in_maps = []
    for core in range(NCORES):
        sl = slice(core * PER, (core + 1) * PER)
        # labels arranged so col t = labels of matmul-tile t under the
        # contiguous-DMA pixel permutation (pixel = chunk_base + p*g + j)
        lshard = labf[sl][perm]
        m = np.ascontiguousarray(
            np.concatenate([iota, lshard], axis=1), dtype=np.float32
        )
        in_maps.append({"fhi": hi[sl], "flo": lo[sl], "meta": m})
    return in_maps


def _combine(results):
    sums = np.zeros((C, A), dtype=np.float64)
    counts = np.zeros((C,), dtype=np.float64)
    for r in results:
        sums += r["sums"].astype(np.float64)
        counts += r["cnt"].sum(axis=0, dtype=np.float64)
    amount = np.where(counts == 0, 1.0, counts)
    mean = (sums / amount[:, None]).astype(np.float32)
    counts32 = counts.astype(np.float32)
    sum_weight = np.broadcast_to(counts32[:, None], (C, A)).copy()
    class_dist = counts32
    return mean, sum_weight, class_dist


def _run(features, labels, ignore_label, trace=False, trace_cores=None):
    nc = _build()
    in_maps = _prep_inmaps(features, labels, ignore_label)
    res = run_bass_kernel_spmd(
        nc, in_maps, list(range(NCORES)), trace=trace, trace_cores=trace_cores
    )
    return _combine(res.results), res


def kernel(features, labels, ignore_label):
    out, _ = _run(features, labels, ignore_label)
    return out
